# revision 50
# baseline (speedup 1.0000x reference)
"""Multi-head attention (BS=4, SEQ=2048, D_MODEL=1024, H=16) on 8 trn2 cores.

Sharding: core c = (batch b = c//2, head-half = c%2). Each core computes the
full attention stack for one batch and 8 of the 16 heads (a 512-wide slice of
the model dim), producing a partial output projection; the host sums the two
partials per batch and adds the (folded) output bias.

Key optimizations over the dense formulation:
  * Mask compaction: the mask zeroes ~half the keys, and a masked key
    contributes exactly 0 to both the softmax numerator and denominator
    (reference: exp(-1e9 - max) underflows to 0).  The host gathers only the
    unmasked keys of K/V (zero-padded to a 128-multiple, kcap=1152 for the
    graded inputs), so S/exp/AV shrink by kcap/SEQ.
  * bv folds out of the device entirely: softmax rows sum to 1, so
    ctx = attn@(V Wv^T) + bv, and out += bv @ Wo^T + bo on the host.
    Padded keys then have v == 0 automatically (no device-side masking).
  * AV matmuls run "flipped": stationary = exp(S^T) 128-query chunk
    (M=128), moving = per-head v columns (N=64) plus the mask column (N=1)
    accumulating the softmax denominator.  All 8 query-chunk accumulators
    pack into one PSUM bank via the 2KB zero-region semantics (first matmul
    start=True zeroes the bank; the others first-touch-replace while the
    bytes are still pending-zero).
  * Normalization is a per-partition tensor_scalar multiply (queries live on
    partitions after the flip); a plain eye-matmul transposes the normalized
    ctx of a head pair back to [e, q] layout for the output projection.
    The transpose outputs pack 4-per-bank into the ctxB rotation so they
    never contend with the projection PSUM banks.
  * Everything on the PE is fp16 (fp32 PSUM accumulation), biases ride the
    DVE PSUM->SBUF drains (per-partition scalars), output partials are fp16.
  * Inputs/weights load via a few wide DMAs (HWDGE descriptor generation is
    serial at ~625ns/DMA and otherwise dominates the cold start).

Device-side math (per core):
  q^T = Wq_half @ Q^T + bq        [512e x 2048q]
  k^T = Wk_half @ Kc^T + bk       [512e x kcap]
  v    = Vc @ Wv_half^T           [kcap x 512e]   (+ mask column)
  S^T  = k_h @ q_h^T              [kcap x 2048q]  per head
  P    = exp(S^T / 8)             (ScalarE, f16)
  ctx[qc] += P[:, qc]^T @ [v_h | m]   (flipped AV, denominator in bank B)
  ctxn = (ctx / denom)^T          (DVE scalar-mul + eye-transpose matmul)
  out_partial = ctxn^T @ Wo_half^T    (host adds halves + bo + bv@Wo^T)
"""

import numpy as np

BS, SEQ, DM, NH, DH = 4, 2048, 1024, 16, 64
EH = DM // 2  # 512 e-dims per core = 8 heads
HPC = 8  # heads per core
NCORES = 8
NET = EH // 128  # 4 e tiles per core
NCT = DM // 128  # 8 contraction (d_model) tiles
NQC = SEQ // 512  # 4 query chunks of 512 for the q projection
DEFAULT_KCAP = 1152  # compacted+padded key count for the graded inputs

_compiled = {}


def _build(kcap=DEFAULT_KCAP, n_iters=1):
    from contextlib import ExitStack

    import concourse.bacc as bacc
    import concourse.mybir as mybir
    import concourse.tile as tile

    f32 = mybir.dt.float32
    f16 = mybir.dt.float16
    EXP = mybir.ActivationFunctionType.Exp

    NKT = kcap // 128  # key tiles
    # K-projection free-dim chunks (PSUM bank holds 512 fp32)
    kchunks = []
    off = 0
    while off < kcap:
        sz = min(512, kcap - off)
        kchunks.append((off, sz))
        off += sz

    nc = bacc.Bacc("TRN2", target_bir_lowering=False, debug=False, num_devices=NCORES)

    QT = nc.dram_tensor("qt", [DM, SEQ], f16, kind="ExternalInput").ap()
    KTC = nc.dram_tensor("ktc", [DM, kcap], f16, kind="ExternalInput").ap()
    VTC = nc.dram_tensor("vtc", [DM, kcap], f16, kind="ExternalInput").ap()
    WQT = nc.dram_tensor("wqt", [DM, EH], f16, kind="ExternalInput").ap()
    WKT = nc.dram_tensor("wkt", [DM, EH], f16, kind="ExternalInput").ap()
    WVT = nc.dram_tensor("wvt", [DM, EH], f16, kind="ExternalInput").ap()
    WOT = nc.dram_tensor("wot", [EH, DM], f16, kind="ExternalInput").ap()
    BQ = nc.dram_tensor("bq", [EH], f32, kind="ExternalInput").ap()
    BK = nc.dram_tensor("bk", [EH], f32, kind="ExternalInput").ap()
    MSKF = nc.dram_tensor("mskf", [kcap], f16, kind="ExternalInput").ap()
    EYE = nc.dram_tensor("eye", [128, 128], f16, kind="ExternalInput").ap()
    OUT = nc.dram_tensor("out", [SEQ, DM], f16, kind="ExternalOutput").ap()

    with tile.TileContext(nc) as tc, ExitStack() as top:
        persist = top.enter_context(tc.tile_pool(name="persist", bufs=1))
        wts = top.enter_context(tc.tile_pool(name="wts", bufs=1))
        att = top.enter_context(tc.tile_pool(name="att", bufs=1))
        obp = top.enter_context(tc.tile_pool(name="obp", bufs=2))
        # PSUM budget (8 banks): psS 2x[128,1024] = 4, ctxA+ctxB = 2, pp 2.
        psS = top.enter_context(tc.tile_pool(name="psS", bufs=2, space="PSUM"))
        psC = top.enter_context(tc.tile_pool(name="psC", bufs=1, space="PSUM"))
        psP = top.enter_context(tc.tile_pool(name="psP", bufs=2, space="PSUM"))

        # persistent activations
        qT = [persist.tile([128, SEQ], f16, tag=f"qT{i}", name=f"qT{i}") for i in range(NET)]
        kT = [persist.tile([128, kcap], f16, tag=f"kT{i}", name=f"kT{i}") for i in range(NET)]
        vones = [
            persist.tile([128, EH + 1], f16, tag=f"vo{i}", name=f"vo{i}")
            for i in range(NKT)
        ]
        ctxn = [persist.tile([128, SEQ], f16, tag=f"cn{i}", name=f"cn{i}") for i in range(NET)]
        bqt = persist.tile([128, NET], f32, tag="bqt")
        bkt = persist.tile([128, NET], f32, tag="bkt")
        mf = persist.tile([128, NKT], f16, tag="mf")
        eye = persist.tile([128, 128], f16, tag="eye")

        # wide staging tiles: few big DMAs (HWDGE gen is 625ns each, serial)
        wkal = wts.tile([128, NCT * EH], f16, tag="wkal")
        wqal = wts.tile([128, NCT * EH], f16, tag="wqal")
        wval = wts.tile([128, NCT * EH], f16, tag="wval")
        xkal = wts.tile([128, NCT * kcap], f16, tag="xkal")
        xqal = wts.tile([128, NCT * SEQ], f16, tag="xqal")
        xval = wts.tile([128, NCT * kcap], f16, tag="xval")
        # wo reuses the front of xkal (dead after the k-projections)
        woal = xkal[:, 0 : NET * DM]

        def _load(dst, src, width, groups):
            # dst col-block c holds src rows [c*128, (c+1)*128); grouped DMAs
            nchunk = src.shape[0] // 128
            per = nchunk // groups
            for g in range(groups):
                nc.sync.dma_start(
                    out=dst[:, g * per * width : (g + 1) * per * width].rearrange(
                        "p (f e) -> p f e", f=per
                    ),
                    in_=src[g * per * 128 : (g + 1) * per * 128, :].rearrange(
                        "(f p) e -> p f e", p=128
                    ),
                )

        def _load_cols(dst, src, width, lo, hi):
            # load columns [lo, hi) of every 128-row chunk of src
            nchunk = src.shape[0] // 128
            nc.sync.dma_start(
                out=dst.rearrange("p (f e) -> p f e", f=nchunk)[:, :, lo:hi],
                in_=src[:, lo:hi].rearrange("(f p) e -> p f e", p=128),
            )

        # DMA issue order is the cold-start critical path (the cost model
        # serializes all DMA transfers).  Loads are column-sliced to match
        # first use: wk/wq arrive one et-slice at a time (kproj/qproj(et)
        # only read their 128-column slice of each contraction chunk), q
        # arrives one 512-column quarter at a time, k/v in key halves.
        khalf = (kcap // 2 + 127) // 128 * 128
        _load_cols(wkal, WKT, EH, 0, 128)
        _load_cols(xkal, KTC, kcap, 0, khalf)
        nc.sync.dma_start(out=bqt, in_=BQ.rearrange("(t p) -> p t", p=128))
        nc.sync.dma_start(out=bkt, in_=BK.rearrange("(t p) -> p t", p=128))
        _load_cols(wqal, WQT, EH, 0, 128)
        _load_cols(xqal, QT, SEQ, 0, 512)
        _load_cols(xqal, QT, SEQ, 512, 1024)
        _load_cols(xkal, KTC, kcap, khalf, kcap)
        nc.sync.dma_start(out=mf, in_=MSKF.rearrange("(t p) -> p t", p=128))
        _load(wval, WVT, EH, 2)
        _load_cols(xval, VTC, kcap, 0, khalf)
        _load_cols(xval, VTC, kcap, khalf, kcap)
        _load_cols(xqal, QT, SEQ, 1024, 1536)
        _load_cols(xqal, QT, SEQ, 1536, 2048)
        nc.sync.dma_start(out=eye, in_=EYE)
        _load_cols(wkal, WKT, EH, 128, 256)
        _load_cols(wqal, WQT, EH, 128, 256)
        _load_cols(wkal, WKT, EH, 256, 512)
        _load_cols(wqal, WQT, EH, 256, 512)
        _load(woal, WOT, DM, 1)

        def qproj(et, scs=range(NQC)):
            for sc in scs:
                pp = psP.tile([128, 512], f32, tag="pp", name=f"ppq{et}{sc}")
                for k in range(NCT):
                    nc.tensor.matmul(
                        pp,
                        wqal[:, k * EH + et * 128 : k * EH + (et + 1) * 128],
                        xqal[:, k * SEQ + sc * 512 : k * SEQ + (sc + 1) * 512],
                        start=(k == 0),
                        stop=(k == NCT - 1),
                    )
                nc.vector.tensor_scalar_add(
                    out=qT[et][:, sc * 512 : (sc + 1) * 512],
                    in0=pp,
                    scalar1=bqt[:, et : et + 1],
                )

        def kproj(et, cis=None):
            for ci, (off, sz) in enumerate(kchunks):
                if cis is not None and ci not in cis:
                    continue
                pp = psP.tile([128, 512], f32, tag="pp", name=f"ppk{et}{ci}")
                for k in range(NCT):
                    nc.tensor.matmul(
                        pp[:, 0:sz],
                        wkal[:, k * EH + et * 128 : k * EH + (et + 1) * 128],
                        xkal[:, k * kcap + off : k * kcap + off + sz],
                        start=(k == 0),
                        stop=(k == NCT - 1),
                    )
                nc.vector.tensor_scalar_add(
                    out=kT[et][:, off : off + sz],
                    in0=pp[:, 0:sz],
                    scalar1=bkt[:, et : et + 1],
                )

        def vproj():
            # highest priority: vones gate the AV matmuls, which gate ex-tile
            # recycling, which gates the exp chain — the serial backbone.
            hp = _ES()
            hp.enter_context(tc.high_priority(offset=2_000_000))
            for st in range(NKT):
                pp = psP.tile([128, 512], f32, tag="pp", name=f"ppv{st}")
                for k in range(NCT):
                    nc.tensor.matmul(
                        pp,
                        xval[:, k * kcap + st * 128 : k * kcap + (st + 1) * 128],
                        wval[:, k * EH : (k + 1) * EH],
                        start=(k == 0),
                        stop=(k == NCT - 1),
                    )
                nc.vector.tensor_copy(out=vones[st][:, 0:EH], in_=pp)
                nc.vector.tensor_copy(
                    out=vones[st][:, EH : EH + 1], in_=mf[:, st : st + 1]
                )
            hp.close()

        from contextlib import ExitStack as _ES

        def emit_half(pr, qh):
            # one q-half of head pair (2*pr, 2*pr+1); cps stages both heads'
            # normalized ctx as [q=128, (qc 8, head-parity 2, e 64)] for the
            # transposes.  qh-major so the qh0 transposes land mid-pair and
            # the output projection's first half can overlap the last pair.
            # The whole half runs at elevated scheduler priority: the exp
            # chain on ScalarE is the serial backbone, so its feeders must
            # always outrank projection/output filler in the ready heaps.
            hp = _ES()
            hp.enter_context(tc.high_priority(offset=1_000_000))
            cps = att.tile([128, 1024], f16, tag=f"cs{qh}", name=f"cs{pr}_{qh}")
            for hh in range(2):
                h = pr * 2 + hh
                ro = hh * 64
                ctxA = psC.tile([128, 512], f32, tag="ctxA", name=f"cA{h}{qh}")
                ctxB = psC.tile([128, 512], f32, tag="ctxB", name=f"cB{h}{qh}")
                for kt in range(NKT):
                    sS = psS.tile([128, 1024], f32, tag="S", name=f"S{h}{qh}{kt}")
                    for q2 in range(2):
                        nc.tensor.matmul(
                            sS[:, q2 * 512 : (q2 + 1) * 512],
                            kT[pr][ro : ro + 64, kt * 128 : (kt + 1) * 128],
                            qT[pr][
                                ro : ro + 64,
                                qh * 1024 + q2 * 512 : qh * 1024 + (q2 + 1) * 512,
                            ],
                            start=True,
                            stop=True,
                        )
                    ex = att.tile(
                        [128, 1024],
                        f16,
                        tag=f"e{kt % 4}",
                        name=f"ex{h}{qh}{kt}",
                        bufs=3,
                    )
                    nc.scalar.activation(out=ex, in_=sS, func=EXP, scale=0.125)
                    first = kt == 0
                    last = kt == NKT - 1
                    for qc in range(8):
                        nc.tensor.matmul(
                            ctxA[:, qc * 64 : (qc + 1) * 64],
                            ex[:, qc * 128 : (qc + 1) * 128],
                            vones[kt][:, h * 64 : (h + 1) * 64],
                            start=(first and qc == 0),
                            stop=(last and qc == 7),
                        )
                        nc.tensor.matmul(
                            ctxB[:, qc : qc + 1],
                            ex[:, qc * 128 : (qc + 1) * 128],
                            vones[kt][:, EH : EH + 1],
                            start=(first and qc == 0),
                            stop=(last and qc == 7),
                        )
                rcp = att.tile([128, 8], f32, tag="rcp", name=f"r{h}{qh}", bufs=2)
                nc.vector.reciprocal(out=rcp, in_=ctxB[:, 0:8])
                cview = cps.rearrange("p (qc hh e) -> p qc hh e", hh=2, e=64)
                for qc in range(8):
                    nc.vector.tensor_scalar_mul(
                        out=cview[:, qc, hh, :],
                        in0=ctxA[:, qc * 64 : (qc + 1) * 64],
                        scalar1=rcp[:, qc : qc + 1],
                    )
            # transposes: 4 per ctxB-rotation bank (start=True once, the
            # rest first-touch-replace on the pending-zero bytes)
            for g in range(2):
                tp = psC.tile([128, 512], f32, tag="ctxB", name=f"tp{pr}{qh}{g}")
                for j in range(4):
                    qc = g * 4 + j
                    nc.tensor.matmul(
                        tp[:, j * 128 : (j + 1) * 128],
                        cps[:, qc * 128 : (qc + 1) * 128],
                        eye,
                        start=(j == 0),
                        stop=(j == 3),
                    )
                for j in range(4):
                    qc = g * 4 + j
                    nc.vector.tensor_copy(
                        out=ctxn[pr][
                            :, qh * 1024 + qc * 128 : qh * 1024 + (qc + 1) * 128
                        ],
                        in_=tp[:, j * 128 : (j + 1) * 128],
                    )
            hp.close()

        def outproj_range(sps, ets=range(NET), partial_in=None, partial_out=None):
            # partial_out: stage the (et subset) accumulation into SBUF f16;
            # partial_in: add the staged partial onto this pass's PSUM result.
            for sp in sps:
                ob = None
                if partial_out is None:
                    ob = obp.tile([128, 2 * DM], f16, tag="ob", name=f"ob{sp}")
                for two in range(2):
                    st = sp * 2 + two
                    for oc in range(2):
                        po = psP.tile([128, 512], f32, tag="pp", name=f"po{st}{oc}")
                        for ei, et in enumerate(ets):
                            nc.tensor.matmul(
                                po,
                                ctxn[et][:, st * 128 : (st + 1) * 128],
                                woal[:, et * DM + oc * 512 : et * DM + (oc + 1) * 512],
                                start=(ei == 0),
                                stop=(ei == len(ets) - 1),
                            )
                        seg = slice(
                            (st - sps[0] * 2) * DM + oc * 512,
                            (st - sps[0] * 2) * DM + (oc + 1) * 512,
                        )
                        if partial_out is not None:
                            nc.vector.tensor_copy(out=partial_out[:, seg], in_=po)
                        elif partial_in is not None:
                            nc.vector.tensor_tensor(
                                out=ob[:, two * DM + oc * 512 : two * DM + (oc + 1) * 512],
                                in0=po,
                                in1=partial_in[:, seg],
                                op=mybir.AluOpType.add,
                            )
                        else:
                            nc.vector.tensor_copy(
                                out=ob[:, two * DM + oc * 512 : two * DM + (oc + 1) * 512],
                                in_=po,
                            )
                if ob is not None:
                    nc.sync.dma_start(
                        out=OUT[sp * 256 : (sp + 1) * 256, :].rearrange(
                            "(two p) dm -> p two dm", p=128
                        ),
                        in_=ob.rearrange("p (two dm) -> p two dm", two=2),
                    )

        for _it in range(n_iters):
            # emission order = scheduler priority: each attention half-pair
            # outranks the projection units it doesn't yet need, which serve
            # as PE gap filler during the ACT-bound stretches; the psP "pp"
            # rotation order matches chronology.
            # staggered half order: qh0 halves run two pairs ahead of qh1
            # halves, so (a) each pair's projections have two half-windows of
            # PE slack to hide in, and (b) the output projection's first half
            # unlocks at half 6 and fills the late windows.
            # emission order respects data deps (producers strictly before
            # consumers — Tile tracks reads against previously-emitted
            # writers only); the high_priority wrap inside emit_half makes
            # the attention chain outrank projection filler independently.
            kproj(0, (0,))
            qproj(0, (0, 1))
            kproj(0, (1, 2))
            vproj()
            qproj(0, (2, 3))
            emit_half(0, 0)
            emit_half(0, 1)
            kproj(1)
            qproj(1, (0, 1))
            emit_half(1, 0)
            kproj(2)
            qproj(2, (0, 1))
            emit_half(2, 0)
            qproj(1, (2, 3))
            emit_half(1, 1)
            kproj(3)
            # wo overwrites the front of xkal — emitted only after the last
            # k-projection has consumed it
            _load(woal, WOT, DM, 1)
            qproj(3, (0, 1))
            emit_half(3, 0)
            qproj(2, (2, 3))
            emit_half(2, 1)
            qproj(3, (2, 3))
            emit_half(3, 1)
            outproj_range(range(SEQ // 256))

    nc.compile()
    return nc


def _get_program(kcap):
    if kcap not in _compiled:
        _compiled[kcap] = _build(kcap)
    return _compiled[kcap]


def kernel(**inputs):
    from concourse.bass_utils import run_bass_kernel_spmd

    Q = np.asarray(inputs["Q"], dtype=np.float32)
    K = np.asarray(inputs["K"], dtype=np.float32)
    V = np.asarray(inputs["V"], dtype=np.float32)
    mask = np.asarray(inputs["mask"], dtype=np.int32)
    Wq = np.asarray(inputs["Wq"], dtype=np.float32)
    Wk = np.asarray(inputs["Wk"], dtype=np.float32)
    Wv = np.asarray(inputs["Wv"], dtype=np.float32)
    Wo = np.asarray(inputs["Wo"], dtype=np.float32)
    bq = np.asarray(inputs["bq"], dtype=np.float32)
    bk = np.asarray(inputs["bk"], dtype=np.float32)
    bv = np.asarray(inputs["bv"], dtype=np.float32)
    bo = np.asarray(inputs["bo"], dtype=np.float32)

    # host-side key compaction: masked keys contribute exactly 0 to both the
    # softmax numerator and denominator, so only the unmasked keys ship to
    # the device (zero-padded to a 128 multiple, same capacity on all cores).
    idxs = [np.nonzero(mask[b, 0, 0, :])[0] for b in range(BS)]
    counts = [len(ix) for ix in idxs]
    kcap = max(128, -(-max(counts) // 128) * 128)

    nc = _get_program(kcap)

    QTs = [np.ascontiguousarray(Q[b].T.astype(np.float16)) for b in range(BS)]
    KTCs, VTCs, MSKs = [], [], []
    for b in range(BS):
        kc = np.zeros((kcap, DM), np.float32)
        vc = np.zeros((kcap, DM), np.float32)
        kc[: counts[b]] = K[b][idxs[b]]
        vc[: counts[b]] = V[b][idxs[b]]
        KTCs.append(np.ascontiguousarray(kc.T.astype(np.float16)))
        VTCs.append(np.ascontiguousarray(vc.T.astype(np.float16)))
        m = np.zeros((kcap,), np.float16)
        m[: counts[b]] = 1
        MSKs.append(m)
    eye = np.eye(128, dtype=np.float16)

    in_maps = []
    for c in range(NCORES):
        b, half = divmod(c, 2)
        sl = slice(half * EH, (half + 1) * EH)
        in_maps.append(
            {
                "qt": QTs[b],
                "ktc": KTCs[b],
                "vtc": VTCs[b],
                "mskf": MSKs[b],
                "wqt": np.ascontiguousarray(Wq[sl, :].T.astype(np.float16)),
                "wkt": np.ascontiguousarray(Wk[sl, :].T.astype(np.float16)),
                "wvt": np.ascontiguousarray(Wv[sl, :].T.astype(np.float16)),
                "wot": np.ascontiguousarray(Wo[:, sl].T.astype(np.float16)),
                "bq": bq[sl],
                "bk": bk[sl],
                "eye": eye,
            }
        )

    trace = bool(int(__import__("os").environ.get("MHA_TRACE", "0")))
    res = run_bass_kernel_spmd(nc, in_maps, list(range(NCORES)), trace=trace)
    kernel.last_results = res

    # host unshard: sum the two half-model partials per batch; bv folds into
    # the output bias because softmax rows sum to 1.
    bo_eff = bo + bv @ Wo.T
    outs = [res.results[c]["out"].astype(np.float32) for c in range(NCORES)]
    out = np.stack(
        [outs[2 * b] + outs[2 * b + 1] + bo_eff[None, :] for b in range(BS)]
    ).astype(np.float32)
    return out


# revision 51
# speedup vs baseline: 1.1724x; 1.1724x over previous
"""Multi-head attention (BS=4, SEQ=2048, D_MODEL=1024, H=16) on 8 trn2 cores.

Sharding: core c = (batch b = c//2, head-half = c%2). Each core computes the
full attention stack for one batch and 8 of the 16 heads (a 512-wide slice of
the model dim), producing a partial output projection; the host sums the two
partials per batch and adds the (folded) output bias.

Key optimizations over the dense formulation:
  * Mask compaction: the mask zeroes ~half the keys, and a masked key
    contributes exactly 0 to both the softmax numerator and denominator
    (reference: exp(-1e9 - max) underflows to 0).  The host gathers only the
    unmasked keys of K/V (zero-padded to a 128-multiple, kcap=1152 for the
    graded inputs), so S/exp/AV shrink by kcap/SEQ.
  * bv folds out of the device entirely: softmax rows sum to 1, so
    ctx = attn@(V Wv^T) + bv, and out += bv @ Wo^T + bo on the host.
    Padded keys then have v == 0 automatically (no device-side masking).
  * AV matmuls run "flipped": stationary = exp(S^T) 128-query chunk
    (M=128), moving = per-head v columns (N=64) plus the mask column (N=1)
    accumulating the softmax denominator.  All 8 query-chunk accumulators
    pack into one PSUM bank via the 2KB zero-region semantics (first matmul
    start=True zeroes the bank; the others first-touch-replace while the
    bytes are still pending-zero).
  * Normalization is a per-partition tensor_scalar multiply (queries live on
    partitions after the flip); a plain eye-matmul transposes the normalized
    ctx of a head pair back to [e, q] layout for the output projection.
    The transpose outputs pack 4-per-bank into the ctxB rotation so they
    never contend with the projection PSUM banks.
  * Everything on the PE is fp16 (fp32 PSUM accumulation), biases ride the
    DVE PSUM->SBUF drains (per-partition scalars), output partials are fp16.
  * Inputs/weights load via a few wide DMAs (HWDGE descriptor generation is
    serial at ~625ns/DMA and otherwise dominates the cold start).

Device-side math (per core):
  q^T = Wq_half @ Q^T + bq        [512e x 2048q]
  k^T = Wk_half @ Kc^T + bk       [512e x kcap]
  v    = Vc @ Wv_half^T           [kcap x 512e]   (+ mask column)
  S^T  = k_h @ q_h^T              [kcap x 2048q]  per head
  P    = exp(S^T / 8)             (ScalarE, f16)
  ctx[qc] += P[:, qc]^T @ [v_h | m]   (flipped AV, denominator in bank B)
  ctxn = (ctx / denom)^T          (DVE scalar-mul + eye-transpose matmul)
  out_partial = ctxn^T @ Wo_half^T    (host adds halves + bo + bv@Wo^T)
"""

import numpy as np

BS, SEQ, DM, NH, DH = 4, 2048, 1024, 16, 64
EH = DM // 2  # 512 e-dims per core = 8 heads
HPC = 8  # heads per core
NCORES = 8
NET = EH // 128  # 4 e tiles per core
NCT = DM // 128  # 8 contraction (d_model) tiles
NQC = SEQ // 512  # 4 query chunks of 512 for the q projection
DEFAULT_KCAP = 1152  # compacted+padded key count for the graded inputs

_compiled = {}


def _build(kcap=DEFAULT_KCAP, n_iters=1):
    from contextlib import ExitStack

    import concourse.bacc as bacc
    import concourse.mybir as mybir
    import concourse.tile as tile

    f32 = mybir.dt.float32
    f16 = mybir.dt.float16
    EXP = mybir.ActivationFunctionType.Exp

    NKT = kcap // 128  # key tiles
    # K-projection free-dim chunks (PSUM bank holds 512 fp32)
    kchunks = []
    off = 0
    while off < kcap:
        sz = min(512, kcap - off)
        kchunks.append((off, sz))
        off += sz

    nc = bacc.Bacc("TRN2", target_bir_lowering=False, debug=False, num_devices=NCORES)

    QT = nc.dram_tensor("qt", [DM, SEQ], f16, kind="ExternalInput").ap()
    KTC = nc.dram_tensor("ktc", [DM, kcap], f16, kind="ExternalInput").ap()
    VTC = nc.dram_tensor("vtc", [DM, kcap], f16, kind="ExternalInput").ap()
    WQT = nc.dram_tensor("wqt", [DM, EH], f16, kind="ExternalInput").ap()
    WKT = nc.dram_tensor("wkt", [DM, EH], f16, kind="ExternalInput").ap()
    WVT = nc.dram_tensor("wvt", [DM, EH], f16, kind="ExternalInput").ap()
    WOT = nc.dram_tensor("wot", [EH, DM], f16, kind="ExternalInput").ap()
    BQ = nc.dram_tensor("bq", [EH], f32, kind="ExternalInput").ap()
    BK = nc.dram_tensor("bk", [EH], f32, kind="ExternalInput").ap()
    MSKF = nc.dram_tensor("mskf", [kcap], f16, kind="ExternalInput").ap()
    EYE = nc.dram_tensor("eye", [128, 128], f16, kind="ExternalInput").ap()
    OUT = nc.dram_tensor("out", [SEQ, DM], f16, kind="ExternalOutput").ap()

    with tile.TileContext(nc) as tc, ExitStack() as top:
        persist = top.enter_context(tc.tile_pool(name="persist", bufs=1))
        wts = top.enter_context(tc.tile_pool(name="wts", bufs=1))
        att = top.enter_context(tc.tile_pool(name="att", bufs=1))
        obp = top.enter_context(tc.tile_pool(name="obp", bufs=2))
        # PSUM budget (8 banks): psS 2x[128,1024] = 4, ctxA+ctxB = 2, pp 2.
        psS = top.enter_context(tc.tile_pool(name="psS", bufs=2, space="PSUM"))
        psC = top.enter_context(tc.tile_pool(name="psC", bufs=1, space="PSUM"))
        psP = top.enter_context(tc.tile_pool(name="psP", bufs=2, space="PSUM"))

        # persistent activations
        qT = [persist.tile([128, SEQ], f16, tag=f"qT{i}", name=f"qT{i}") for i in range(NET)]
        kT = [persist.tile([128, kcap], f16, tag=f"kT{i}", name=f"kT{i}") for i in range(NET)]
        vones = [
            persist.tile([128, EH + 1], f16, tag=f"vo{i}", name=f"vo{i}")
            for i in range(NKT)
        ]
        ctxn = [persist.tile([128, SEQ], f16, tag=f"cn{i}", name=f"cn{i}") for i in range(NET)]
        bqt = persist.tile([128, NET], f32, tag="bqt")
        bkt = persist.tile([128, NET], f32, tag="bkt")
        mf = persist.tile([128, NKT], f16, tag="mf")
        eye = persist.tile([128, 128], f16, tag="eye")

        # wide staging tiles: few big DMAs (HWDGE gen is 625ns each, serial)
        wkal = wts.tile([128, NCT * EH], f16, tag="wkal")
        wqal = wts.tile([128, NCT * EH], f16, tag="wqal")
        wval = wts.tile([128, NCT * EH], f16, tag="wval")
        woal = wts.tile([128, NET * DM], f16, tag="woal")
        xkal = wts.tile([128, NCT * kcap], f16, tag="xkal")
        xqal = wts.tile([128, NCT * SEQ], f16, tag="xqal")
        xval = wts.tile([128, NCT * kcap], f16, tag="xval")

        def _load(dst, src, width, groups):
            # dst col-block c holds src rows [c*128, (c+1)*128); grouped DMAs
            nchunk = src.shape[0] // 128
            per = nchunk // groups
            for g in range(groups):
                nc.sync.dma_start(
                    out=dst[:, g * per * width : (g + 1) * per * width].rearrange(
                        "p (f e) -> p f e", f=per
                    ),
                    in_=src[g * per * 128 : (g + 1) * per * 128, :].rearrange(
                        "(f p) e -> p f e", p=128
                    ),
                )

        def _load_cols(dst, src, width, lo, hi):
            # load columns [lo, hi) of every 128-row chunk of src
            nchunk = src.shape[0] // 128
            nc.sync.dma_start(
                out=dst.rearrange("p (f e) -> p f e", f=nchunk)[:, :, lo:hi],
                in_=src[:, lo:hi].rearrange("(f p) e -> p f e", p=128),
            )

        # DMA issue order is the cold-start critical path (the cost model
        # serializes all DMA transfers).  Loads are column-sliced to match
        # first use: wk/wq arrive one et-slice at a time (kproj/qproj(et)
        # only read their 128-column slice of each contraction chunk), q
        # arrives one 512-column quarter at a time, k/v in key halves.
        khalf = (kcap // 2 + 127) // 128 * 128
        _load_cols(wkal, WKT, EH, 0, 128)
        _load_cols(xkal, KTC, kcap, 0, khalf)
        nc.sync.dma_start(out=bqt, in_=BQ.rearrange("(t p) -> p t", p=128))
        nc.sync.dma_start(out=bkt, in_=BK.rearrange("(t p) -> p t", p=128))
        _load_cols(wqal, WQT, EH, 0, 128)
        _load_cols(xqal, QT, SEQ, 0, 512)
        _load_cols(xqal, QT, SEQ, 512, 1024)
        _load_cols(xkal, KTC, kcap, khalf, kcap)
        nc.sync.dma_start(out=mf, in_=MSKF.rearrange("(t p) -> p t", p=128))
        _load(wval, WVT, EH, 2)
        _load_cols(xval, VTC, kcap, 0, khalf)
        _load_cols(xval, VTC, kcap, khalf, kcap)
        _load_cols(xqal, QT, SEQ, 1024, 1536)
        _load_cols(xqal, QT, SEQ, 1536, 2048)
        nc.sync.dma_start(out=eye, in_=EYE)
        _load_cols(wkal, WKT, EH, 128, 256)
        _load_cols(wqal, WQT, EH, 128, 256)
        _load_cols(wkal, WKT, EH, 256, 512)
        _load_cols(wqal, WQT, EH, 256, 512)
        _load(woal, WOT, DM, 1)

        def qproj(et, scs=range(NQC)):
            for sc in scs:
                pp = psP.tile([128, 512], f32, tag="pp", name=f"ppq{et}{sc}")
                for k in range(NCT):
                    nc.tensor.matmul(
                        pp,
                        wqal[:, k * EH + et * 128 : k * EH + (et + 1) * 128],
                        xqal[:, k * SEQ + sc * 512 : k * SEQ + (sc + 1) * 512],
                        start=(k == 0),
                        stop=(k == NCT - 1),
                    )
                nc.vector.tensor_scalar_add(
                    out=qT[et][:, sc * 512 : (sc + 1) * 512],
                    in0=pp,
                    scalar1=bqt[:, et : et + 1],
                )

        def kproj(et, cis=None):
            for ci, (off, sz) in enumerate(kchunks):
                if cis is not None and ci not in cis:
                    continue
                pp = psP.tile([128, 512], f32, tag="pp", name=f"ppk{et}{ci}")
                for k in range(NCT):
                    nc.tensor.matmul(
                        pp[:, 0:sz],
                        wkal[:, k * EH + et * 128 : k * EH + (et + 1) * 128],
                        xkal[:, k * kcap + off : k * kcap + off + sz],
                        start=(k == 0),
                        stop=(k == NCT - 1),
                    )
                nc.vector.tensor_scalar_add(
                    out=kT[et][:, off : off + sz],
                    in0=pp[:, 0:sz],
                    scalar1=bkt[:, et : et + 1],
                )

        def vproj():
            # highest priority: vones gate the AV matmuls, which gate ex-tile
            # recycling, which gates the exp chain — the serial backbone.
            hp = _ES()
            hp.enter_context(tc.high_priority(offset=2_000_000))
            for st in range(NKT):
                pp = psP.tile([128, 512], f32, tag="pp", name=f"ppv{st}")
                for k in range(NCT):
                    nc.tensor.matmul(
                        pp,
                        xval[:, k * kcap + st * 128 : k * kcap + (st + 1) * 128],
                        wval[:, k * EH : (k + 1) * EH],
                        start=(k == 0),
                        stop=(k == NCT - 1),
                    )
                nc.vector.tensor_copy(out=vones[st][:, 0:EH], in_=pp)
                nc.vector.tensor_copy(
                    out=vones[st][:, EH : EH + 1], in_=mf[:, st : st + 1]
                )
            hp.close()

        from contextlib import ExitStack as _ES

        def emit_half(pr, qh):
            # one q-half of head pair (2*pr, 2*pr+1); cps stages both heads'
            # normalized ctx as [q=128, (qc 8, head-parity 2, e 64)] for the
            # transposes.  qh-major so the qh0 transposes land mid-pair and
            # the output projection's first half can overlap the last pair.
            # The whole half runs at elevated scheduler priority: the exp
            # chain on ScalarE is the serial backbone, so its feeders must
            # always outrank projection/output filler in the ready heaps.
            hp = _ES()
            hp.enter_context(tc.high_priority(offset=1_000_000))
            cps = att.tile([128, 1024], f16, tag=f"cs{qh}", name=f"cs{pr}_{qh}")
            for hh in range(2):
                h = pr * 2 + hh
                ro = hh * 64
                ctxA = psC.tile([128, 512], f32, tag="ctxA", name=f"cA{h}{qh}")
                ctxB = psC.tile([128, 512], f32, tag="ctxB", name=f"cB{h}{qh}")
                for kt in range(NKT):
                    sS = psS.tile([128, 1024], f32, tag="S", name=f"S{h}{qh}{kt}")
                    for q2 in range(2):
                        nc.tensor.matmul(
                            sS[:, q2 * 512 : (q2 + 1) * 512],
                            kT[pr][ro : ro + 64, kt * 128 : (kt + 1) * 128],
                            qT[pr][
                                ro : ro + 64,
                                qh * 1024 + q2 * 512 : qh * 1024 + (q2 + 1) * 512,
                            ],
                            start=True,
                            stop=True,
                        )
                    ex = att.tile(
                        [128, 1024],
                        f16,
                        tag=f"e{kt % 4}",
                        name=f"ex{h}{qh}{kt}",
                        bufs=3,
                    )
                    nc.scalar.activation(out=ex, in_=sS, func=EXP, scale=0.125)
                    first = kt == 0
                    last = kt == NKT - 1
                    for qc in range(8):
                        nc.tensor.matmul(
                            ctxA[:, qc * 64 : (qc + 1) * 64],
                            ex[:, qc * 128 : (qc + 1) * 128],
                            vones[kt][:, h * 64 : (h + 1) * 64],
                            start=(first and qc == 0),
                            stop=(last and qc == 7),
                        )
                        nc.tensor.matmul(
                            ctxB[:, qc : qc + 1],
                            ex[:, qc * 128 : (qc + 1) * 128],
                            vones[kt][:, EH : EH + 1],
                            start=(first and qc == 0),
                            stop=(last and qc == 7),
                        )
                rcp = att.tile([128, 8], f32, tag="rcp", name=f"r{h}{qh}", bufs=2)
                nc.vector.reciprocal(out=rcp, in_=ctxB[:, 0:8])
                cview = cps.rearrange("p (qc hh e) -> p qc hh e", hh=2, e=64)
                for qc in range(8):
                    nc.vector.tensor_scalar_mul(
                        out=cview[:, qc, hh, :],
                        in0=ctxA[:, qc * 64 : (qc + 1) * 64],
                        scalar1=rcp[:, qc : qc + 1],
                    )
            # transposes: 4 per ctxB-rotation bank (start=True once, the
            # rest first-touch-replace on the pending-zero bytes)
            for g in range(2):
                tp = psC.tile([128, 512], f32, tag="ctxB", name=f"tp{pr}{qh}{g}")
                for j in range(4):
                    qc = g * 4 + j
                    nc.tensor.matmul(
                        tp[:, j * 128 : (j + 1) * 128],
                        cps[:, qc * 128 : (qc + 1) * 128],
                        eye,
                        start=(j == 0),
                        stop=(j == 3),
                    )
                for j in range(4):
                    qc = g * 4 + j
                    nc.vector.tensor_copy(
                        out=ctxn[pr][
                            :, qh * 1024 + qc * 128 : qh * 1024 + (qc + 1) * 128
                        ],
                        in_=tp[:, j * 128 : (j + 1) * 128],
                    )
            hp.close()

        def outproj_range(sps, ets=range(NET), partial_in=None, partial_out=None):
            # partial_out: stage the (et subset) accumulation into SBUF f16;
            # partial_in: add the staged partial onto this pass's PSUM result.
            for sp in sps:
                ob = None
                if partial_out is None:
                    ob = obp.tile([128, 2 * DM], f16, tag="ob", name=f"ob{sp}")
                for two in range(2):
                    st = sp * 2 + two
                    for oc in range(2):
                        po = psP.tile([128, 512], f32, tag="pp", name=f"po{st}{oc}")
                        for ei, et in enumerate(ets):
                            nc.tensor.matmul(
                                po,
                                ctxn[et][:, st * 128 : (st + 1) * 128],
                                woal[:, et * DM + oc * 512 : et * DM + (oc + 1) * 512],
                                start=(ei == 0),
                                stop=(ei == len(ets) - 1),
                            )
                        seg = slice(
                            (st - sps[0] * 2) * DM + oc * 512,
                            (st - sps[0] * 2) * DM + (oc + 1) * 512,
                        )
                        if partial_out is not None:
                            nc.vector.tensor_copy(out=partial_out[:, seg], in_=po)
                        elif partial_in is not None:
                            nc.vector.tensor_tensor(
                                out=ob[:, two * DM + oc * 512 : two * DM + (oc + 1) * 512],
                                in0=po,
                                in1=partial_in[:, seg],
                                op=mybir.AluOpType.add,
                            )
                        else:
                            nc.vector.tensor_copy(
                                out=ob[:, two * DM + oc * 512 : two * DM + (oc + 1) * 512],
                                in_=po,
                            )
                if ob is not None:
                    nc.sync.dma_start(
                        out=OUT[sp * 256 : (sp + 1) * 256, :].rearrange(
                            "(two p) dm -> p two dm", p=128
                        ),
                        in_=ob.rearrange("p (two dm) -> p two dm", two=2),
                    )

        for _it in range(n_iters):
            # emission order = scheduler priority: each attention half-pair
            # outranks the projection units it doesn't yet need, which serve
            # as PE gap filler during the ACT-bound stretches; the psP "pp"
            # rotation order matches chronology.
            # staggered half order: qh0 halves run two pairs ahead of qh1
            # halves, so (a) each pair's projections have two half-windows of
            # PE slack to hide in, and (b) the output projection's first half
            # unlocks at half 6 and fills the late windows.
            # emission order respects data deps (producers strictly before
            # consumers — Tile tracks reads against previously-emitted
            # writers only); the high_priority wrap inside emit_half makes
            # the attention chain outrank projection filler independently.
            kproj(0, (0,))
            qproj(0, (0, 1))
            kproj(0, (1, 2))
            vproj()
            qproj(0, (2, 3))
            emit_half(0, 0)
            emit_half(0, 1)
            kproj(1)
            qproj(1, (0, 1))
            emit_half(1, 0)
            kproj(2)
            qproj(2, (0, 1))
            emit_half(2, 0)
            qproj(1, (2, 3))
            emit_half(1, 1)
            kproj(3)
            _load(woal, WOT, DM, 1)
            qproj(3, (0, 1))
            emit_half(3, 0)
            qproj(2, (2, 3))
            emit_half(2, 1)
            qproj(3, (2, 3))
            emit_half(3, 1)
            outproj_range(range(SEQ // 256))

    nc.compile()
    return nc


def _get_program(kcap):
    if kcap not in _compiled:
        _compiled[kcap] = _build(kcap)
    return _compiled[kcap]


def kernel(**inputs):
    from concourse.bass_utils import run_bass_kernel_spmd

    Q = np.asarray(inputs["Q"], dtype=np.float32)
    K = np.asarray(inputs["K"], dtype=np.float32)
    V = np.asarray(inputs["V"], dtype=np.float32)
    mask = np.asarray(inputs["mask"], dtype=np.int32)
    Wq = np.asarray(inputs["Wq"], dtype=np.float32)
    Wk = np.asarray(inputs["Wk"], dtype=np.float32)
    Wv = np.asarray(inputs["Wv"], dtype=np.float32)
    Wo = np.asarray(inputs["Wo"], dtype=np.float32)
    bq = np.asarray(inputs["bq"], dtype=np.float32)
    bk = np.asarray(inputs["bk"], dtype=np.float32)
    bv = np.asarray(inputs["bv"], dtype=np.float32)
    bo = np.asarray(inputs["bo"], dtype=np.float32)

    # host-side key compaction: masked keys contribute exactly 0 to both the
    # softmax numerator and denominator, so only the unmasked keys ship to
    # the device (zero-padded to a 128 multiple, same capacity on all cores).
    idxs = [np.nonzero(mask[b, 0, 0, :])[0] for b in range(BS)]
    counts = [len(ix) for ix in idxs]
    kcap = max(128, -(-max(counts) // 128) * 128)

    nc = _get_program(kcap)

    QTs = [np.ascontiguousarray(Q[b].T.astype(np.float16)) for b in range(BS)]
    KTCs, VTCs, MSKs = [], [], []
    for b in range(BS):
        kc = np.zeros((kcap, DM), np.float32)
        vc = np.zeros((kcap, DM), np.float32)
        kc[: counts[b]] = K[b][idxs[b]]
        vc[: counts[b]] = V[b][idxs[b]]
        KTCs.append(np.ascontiguousarray(kc.T.astype(np.float16)))
        VTCs.append(np.ascontiguousarray(vc.T.astype(np.float16)))
        m = np.zeros((kcap,), np.float16)
        m[: counts[b]] = 1
        MSKs.append(m)
    eye = np.eye(128, dtype=np.float16)

    in_maps = []
    for c in range(NCORES):
        b, half = divmod(c, 2)
        sl = slice(half * EH, (half + 1) * EH)
        in_maps.append(
            {
                "qt": QTs[b],
                "ktc": KTCs[b],
                "vtc": VTCs[b],
                "mskf": MSKs[b],
                "wqt": np.ascontiguousarray(Wq[sl, :].T.astype(np.float16)),
                "wkt": np.ascontiguousarray(Wk[sl, :].T.astype(np.float16)),
                "wvt": np.ascontiguousarray(Wv[sl, :].T.astype(np.float16)),
                "wot": np.ascontiguousarray(Wo[:, sl].T.astype(np.float16)),
                "bq": bq[sl],
                "bk": bk[sl],
                "eye": eye,
            }
        )

    trace = bool(int(__import__("os").environ.get("MHA_TRACE", "0")))
    res = run_bass_kernel_spmd(nc, in_maps, list(range(NCORES)), trace=trace)
    kernel.last_results = res

    # host unshard: sum the two half-model partials per batch; bv folds into
    # the output bias because softmax rows sum to 1.
    bo_eff = bo + bv @ Wo.T
    outs = [res.results[c]["out"].astype(np.float32) for c in range(NCORES)]
    out = np.stack(
        [outs[2 * b] + outs[2 * b + 1] + bo_eff[None, :] for b in range(BS)]
    ).astype(np.float32)
    return out
